# revision 4
# baseline (speedup 1.0000x reference)
"""Trainium2 Bass kernel for nn_Attention_43181601194684.

Reference computation:
    h_last  = hidden[0, 1]                          # [B, H]
    proj    = einsum('blh,oh->blo', enc, W) + b     # [B, L, H]
    energies= einsum('bh,blh->bl', h_last, proj)    # [B, L]
    out     = softmax(energies, axis=1)[:, None, :] # [B, 1, L]

Algebraic simplification used here:
    energies[b, l] = (h_last[b] @ W) . enc[b, l] + (h_last[b] . bias)
The per-batch constant cancels inside the softmax, so the device kernel
computes   e[b, l] = v[b] . enc[b, l]   with v = h_last @ W, followed by a
numerically-stable softmax over l.

Sharding: data-parallel over batch. 32 batches / 8 cores = 4 batches per
core; W is replicated; the [2,2,32,512] hidden tensor is sliced to the
4 x 512 h_last rows each core needs (passed pre-transposed as [512, 4]).
"""

import numpy as np

B, L, H = 32, 4096, 512
N_CORES = 8
B_LOC = B // N_CORES  # 4
P = 128               # SBUF partitions
JCH = 4               # 128-row l-blocks per DMA chunk (1 MiB per DMA)
NCH = L // (P * JCH)  # 8 chunks per batch
NCOL = L // P         # 32 energy columns per batch

_PROGRAM = None


def _build_program():
    """Build + compile the single-core Bass/Tile program (SPMD across 8 cores)."""
    from contextlib import ExitStack

    import concourse.bacc as bacc
    import concourse.mybir as mybir
    import concourse.tile as tile
    from concourse.masks import make_identity

    fp32 = mybir.dt.float32
    Alu = mybir.AluOpType
    Act = mybir.ActivationFunctionType

    nc = bacc.Bacc("TRN2", target_bir_lowering=False, debug=False,
                   num_devices=N_CORES)

    enc = nc.dram_tensor("enc", [B_LOC, L, H], fp32, kind="ExternalInput")
    hT = nc.dram_tensor("hT", [H, B_LOC], fp32, kind="ExternalInput")
    Wd = nc.dram_tensor("W", [H, H], fp32, kind="ExternalInput")
    probs = nc.dram_tensor("probs", [B_LOC, L], fp32, kind="ExternalOutput")

    with tile.TileContext(nc) as tc, ExitStack() as ctx:
        consts = ctx.enter_context(tc.tile_pool(name="consts", bufs=1))
        wpool = ctx.enter_context(tc.tile_pool(name="wpool", bufs=1))
        epool = ctx.enter_context(tc.tile_pool(name="epool", bufs=3))
        scratch = ctx.enter_context(tc.tile_pool(name="scratch", bufs=2))
        epers = ctx.enter_context(tc.tile_pool(name="epers", bufs=1))
        small = ctx.enter_context(tc.tile_pool(name="small", bufs=2))
        psum = ctx.enter_context(tc.tile_pool(name="psum", bufs=2, space="PSUM"))

        identity = consts.tile([P, P], fp32, tag="identity")
        make_identity(nc, identity)
        ones_row = consts.tile([1, P], fp32, tag="ones_row")  # bcast lhsT
        nc.vector.memset(ones_row[:], 1.0)
        ones_col = consts.tile([P, 1], fp32, tag="ones_col")  # partition-sum lhsT
        nc.vector.memset(ones_col[:], 1.0)

        # ---- v = h_last @ W  (PE, contraction over o in 4 chunks of 128) ----
        w_r = Wd.rearrange("(c p) h -> c p h", p=P)
        h_r = hT.rearrange("(c p) b -> c p b", p=P)
        w_tiles, h_tiles = [], []
        for c in range(4):
            wt = wpool.tile([P, H], fp32, tag=f"w{c}")
            nc.sync.dma_start(wt[:], w_r[c])
            w_tiles.append(wt)
            ht = wpool.tile([P, B_LOC], fp32, tag=f"h{c}")
            nc.sync.dma_start(ht[:], h_r[c])
            h_tiles.append(ht)

        v_ps = psum.tile([B_LOC, H], fp32, tag="mm_ps")
        for c in range(4):
            nc.tensor.matmul(v_ps[:], h_tiles[c][:], w_tiles[c][:],
                             start=(c == 0), stop=(c == 3))
        v_sb = wpool.tile([B_LOC, H], fp32, tag="v_sb")
        nc.scalar.copy(v_sb[:], v_ps[:])

        # ---- broadcast v[b] to all 128 partitions via one-hot PE matmul ----
        # lhsT[k, m] = (k == bi)  =>  out[m, :] = v_sb[bi, :] for every m
        v_bc = []
        for bi in range(B_LOC):
            sel = consts.tile([B_LOC, P], fp32, tag=f"sel{bi}")
            nc.gpsimd.memset(sel[:], 0.0)
            # iota = 1*partition + 0*free - bi; != 0 keeps 0.0, == 0 fills 1.0
            nc.gpsimd.affine_select(
                out=sel[:], in_=sel[:], compare_op=Alu.not_equal, fill=1.0,
                base=-bi, pattern=[[0, P]], channel_multiplier=1,
            )
            vb_ps = psum.tile([P, H], fp32, tag="mm_ps")
            nc.tensor.matmul(vb_ps[:], sel[:], v_sb[:],
                             start=True, stop=True)
            vb = wpool.tile([P, H], fp32, tag=f"vbc{bi}")
            nc.scalar.copy(vb[:], vb_ps[:])
            v_bc.append(vb)

        # ---- main stream: energies via fused multiply+row-reduce on DVE ----
        # l = c*512 + j*128 + p  ->  column m = c*4 + j, partition p
        enc_r = enc.rearrange("b (c j p) h -> b c p j h", p=P, j=JCH)

        for bi in range(B_LOC):
            e_sb = epers.tile([P, NCOL], fp32, tag=f"e{bi}")
            for c in range(NCH):
                et = epool.tile([P, JCH, H], fp32, tag="et")
                nc.sync.dma_start(et[:], enc_r[bi, c])
                for j in range(JCH):
                    m = c * JCH + j
                    sc = scratch.tile([P, H], fp32, tag="ttr")
                    # fused (enc * v) + row-sum in one native DVE op:
                    # out = (in0 * 1.0) * in1 ; accum_out = row_sum(out)
                    nc.vector.scalar_tensor_tensor(
                        out=sc[:], in0=et[:, j, :], scalar=1.0,
                        in1=v_bc[bi][:],
                        op0=Alu.mult, op1=Alu.mult,
                        accum_out=e_sb[:, m:m + 1],
                    )

            # ---- softmax over the 4096 energies of batch bi ----
            mx = small.tile([P, 1], fp32, tag="mx")
            nc.vector.tensor_reduce(mx[:], e_sb[:], axis=mybir.AxisListType.X,
                                    op=Alu.max)
            mxT_ps = psum.tile([1, P], fp32, tag="red_ps")
            nc.tensor.transpose(mxT_ps[:], mx[:], identity[:])
            ngmax = small.tile([1, 1], fp32, tag="ngmax")
            nc.vector.tensor_reduce(ngmax[:], mxT_ps[:],
                                    axis=mybir.AxisListType.X, op=Alu.max,
                                    negate=True)
            nb_ps = psum.tile([P, 1], fp32, tag="bc_ps")
            nc.tensor.matmul(nb_ps[:], ones_row[:], ngmax[:],
                             start=True, stop=True)
            nbias = small.tile([P, 1], fp32, tag="nbias")
            nc.scalar.copy(nbias[:], nb_ps[:])

            p_sb = epers.tile([P, NCOL], fp32, tag=f"p{bi}")
            ssum = small.tile([P, 1], fp32, tag="ssum")
            nc.scalar.activation(p_sb[:], e_sb[:], Act.Exp,
                                 bias=nbias[:], scale=1.0, accum_out=ssum[:])

            tot_ps = psum.tile([1, 1], fp32, tag="red_ps")
            nc.tensor.matmul(tot_ps[:], ones_col[:], ssum[:],
                             start=True, stop=True)
            rinv = small.tile([1, 1], fp32, tag="rinv")
            nc.vector.reciprocal(rinv[:], tot_ps[:])
            r_ps = psum.tile([P, 1], fp32, tag="bc_ps")
            nc.tensor.matmul(r_ps[:], ones_row[:], rinv[:],
                             start=True, stop=True)
            rbc = small.tile([P, 1], fp32, tag="rbc")
            nc.scalar.copy(rbc[:], r_ps[:])

            o_sb = epers.tile([P, NCOL], fp32, tag=f"o{bi}")
            nc.scalar.mul(o_sb[:], p_sb[:], rbc[:])

            oT_ps = psum.tile([NCOL, P], fp32, tag="oT_ps")
            nc.tensor.transpose(oT_ps[:], o_sb[:], identity[:])
            oT = small.tile([NCOL, P], fp32, tag=f"oT{bi}")
            nc.scalar.copy(oT[:], oT_ps[:])
            nc.sync.dma_start(probs[bi].rearrange("(m p) -> m p", p=P), oT[:])

    nc.compile()
    return nc


def _get_program():
    global _PROGRAM
    if _PROGRAM is None:
        _PROGRAM = _build_program()
    return _PROGRAM


def kernel(hidden, encoder_outputs, W, b):
    """Full-input entry point: shards across 8 NeuronCores, returns [B,1,L]."""
    from concourse.bass_utils import run_bass_kernel_spmd

    hidden = np.asarray(hidden, dtype=np.float32)
    enc = np.asarray(encoder_outputs, dtype=np.float32)
    W = np.asarray(W, dtype=np.float32)

    h_last = hidden[0, 1]  # == hidden[0].transpose(1,0,2)[:, -1, :], [B, H]

    nc = _get_program()
    in_maps = []
    for core in range(N_CORES):
        b0 = core * B_LOC
        in_maps.append({
            "enc": np.ascontiguousarray(enc[b0:b0 + B_LOC]),
            "hT": np.ascontiguousarray(h_last[b0:b0 + B_LOC].T),
            "W": np.ascontiguousarray(W),
        })

    res = run_bass_kernel_spmd(nc, in_maps, list(range(N_CORES)))
    out = np.concatenate([res.results[i]["probs"] for i in range(N_CORES)], axis=0)
    return out[:, None, :].astype(np.float32)


# revision 9
# speedup vs baseline: 1.0093x; 1.0093x over previous
"""Trainium2 Bass kernel for nn_Attention_43181601194684.

Reference computation:
    h_last  = hidden[0, 1]                          # [B, H]
    proj    = einsum('blh,oh->blo', enc, W) + b     # [B, L, H]
    energies= einsum('bh,blh->bl', h_last, proj)    # [B, L]
    out     = softmax(energies, axis=1)[:, None, :] # [B, 1, L]

Algebraic simplification used here:
    energies[b, l] = (h_last[b] @ W) . enc[b, l] + (h_last[b] . bias)
The per-batch constant cancels inside the softmax, so the device kernel
computes   e[b, l] = v[b] . enc[b, l]   with v = h_last @ W, followed by a
numerically-stable softmax over l.

Sharding: data-parallel over batch. 32 batches / 8 cores = 4 batches per
core; W is replicated; the [2,2,32,512] hidden tensor is sliced to the
4 x 512 h_last rows each core needs (passed pre-transposed as [512, 4]).
"""

import numpy as np

B, L, H = 32, 4096, 512
N_CORES = 8
B_LOC = B // N_CORES  # 4
P = 128               # SBUF partitions
JCH = 4               # 128-row l-blocks per DMA chunk (1 MiB per DMA)
NCH = L // (P * JCH)  # 8 chunks per batch
NCOL = L // P         # 32 energy columns per batch

_PROGRAM = None


def _build_program():
    """Build + compile the single-core Bass/Tile program (SPMD across 8 cores)."""
    from contextlib import ExitStack

    import concourse.bacc as bacc
    import concourse.mybir as mybir
    import concourse.tile as tile
    from concourse.masks import make_identity

    fp32 = mybir.dt.float32
    Alu = mybir.AluOpType
    Act = mybir.ActivationFunctionType

    nc = bacc.Bacc("TRN2", target_bir_lowering=False, debug=False,
                   num_devices=N_CORES)

    enc = nc.dram_tensor("enc", [B_LOC, L, H], fp32, kind="ExternalInput")
    hT = nc.dram_tensor("hT", [H, B_LOC], fp32, kind="ExternalInput")
    Wd = nc.dram_tensor("W", [H, H], fp32, kind="ExternalInput")
    probs = nc.dram_tensor("probs", [B_LOC, L], fp32, kind="ExternalOutput")

    with tile.TileContext(nc) as tc, ExitStack() as ctx:
        consts = ctx.enter_context(tc.tile_pool(name="consts", bufs=1))
        wpool = ctx.enter_context(tc.tile_pool(name="wpool", bufs=1))
        epool = ctx.enter_context(tc.tile_pool(name="epool", bufs=4))
        scratch = ctx.enter_context(tc.tile_pool(name="scratch", bufs=2))
        epers = ctx.enter_context(tc.tile_pool(name="epers", bufs=1))
        small = ctx.enter_context(tc.tile_pool(name="small", bufs=2))
        psum = ctx.enter_context(tc.tile_pool(name="psum", bufs=2, space="PSUM"))

        identity = consts.tile([P, P], fp32, tag="identity")
        make_identity(nc, identity)
        ones_row = consts.tile([1, P], fp32, tag="ones_row")  # bcast lhsT
        nc.vector.memset(ones_row[:], 1.0)
        ones_col = consts.tile([P, 1], fp32, tag="ones_col")  # partition-sum lhsT
        nc.vector.memset(ones_col[:], 1.0)

        # ---- v = h_last @ W  (PE, contraction over o in 4 chunks of 128) ----
        # W/hT ride the scalar HWDGE ring so the sync ring is enc-only.
        w_r = Wd.rearrange("(c p) h -> c p h", p=P)
        h_r = hT.rearrange("(c p) b -> c p b", p=P)
        w_tiles, h_tiles = [], []
        for c in range(4):
            wt = wpool.tile([P, H], fp32, tag=f"w{c}")
            nc.scalar.dma_start(wt[:], w_r[c])
            w_tiles.append(wt)
            ht = wpool.tile([P, B_LOC], fp32, tag=f"h{c}")
            nc.scalar.dma_start(ht[:], h_r[c])
            h_tiles.append(ht)

        v_ps = psum.tile([B_LOC, H], fp32, tag="mm_ps")
        for c in range(4):
            nc.tensor.matmul(v_ps[:], h_tiles[c][:], w_tiles[c][:],
                             start=(c == 0), stop=(c == 3))
        v_sb = wpool.tile([B_LOC, H], fp32, tag="v_sb")
        nc.scalar.copy(v_sb[:], v_ps[:])

        # ---- broadcast v[b] to all 128 partitions via one-hot PE matmul ----
        # lhsT[k, m] = (k == bi)  =>  out[m, :] = v_sb[bi, :] for every m
        v_bc = []
        for bi in range(B_LOC):
            sel = consts.tile([B_LOC, P], fp32, tag=f"sel{bi}")
            nc.gpsimd.memset(sel[:], 0.0)
            # iota = 1*partition + 0*free - bi; != 0 keeps 0.0, == 0 fills 1.0
            nc.gpsimd.affine_select(
                out=sel[:], in_=sel[:], compare_op=Alu.not_equal, fill=1.0,
                base=-bi, pattern=[[0, P]], channel_multiplier=1,
            )
            vb_ps = psum.tile([P, H], fp32, tag="mm_ps")
            nc.tensor.matmul(vb_ps[:], sel[:], v_sb[:],
                             start=True, stop=True)
            vb = wpool.tile([P, H], fp32, tag=f"vbc{bi}")
            nc.scalar.copy(vb[:], vb_ps[:])
            v_bc.append(vb)

        # ---- main stream: energies via fused multiply+row-reduce on DVE ----
        # l = c*512 + p*4 + k: partition p owns 4 consecutive rows = one
        # contiguous 8 KiB DRAM run per partition -> long DMA descriptors.
        enc_r = enc.rearrange("b (c p k) h -> b c p k h", p=P, k=JCH)

        for bi in range(B_LOC):
            e_sb = epers.tile([P, NCOL], fp32, tag=f"e{bi}")
            for c in range(NCH):
                et = epool.tile([P, JCH, H], fp32, tag="et")
                nc.sync.dma_start(et[:], enc_r[bi, c])
                for k in range(JCH):
                    m = c * JCH + k
                    sc = scratch.tile([P, H], fp32, tag="ttr")
                    # fused (enc * v) + row-sum in one native DVE op:
                    # out = (in0 * 1.0) * in1 ; accum_out = row_sum(out)
                    nc.vector.scalar_tensor_tensor(
                        out=sc[:], in0=et[:, k, :], scalar=1.0,
                        in1=v_bc[bi][:],
                        op0=Alu.mult, op1=Alu.mult,
                        accum_out=e_sb[:, m:m + 1],
                    )

            # ---- softmax over the 4096 energies of batch bi ----
            mx = small.tile([P, 1], fp32, tag="mx")
            nc.vector.tensor_reduce(mx[:], e_sb[:], axis=mybir.AxisListType.X,
                                    op=Alu.max)
            mxT_ps = psum.tile([1, P], fp32, tag="red_ps")
            nc.tensor.transpose(mxT_ps[:], mx[:], identity[:])
            ngmax = small.tile([1, 1], fp32, tag="ngmax")
            nc.vector.tensor_reduce(ngmax[:], mxT_ps[:],
                                    axis=mybir.AxisListType.X, op=Alu.max,
                                    negate=True)
            nb_ps = psum.tile([P, 1], fp32, tag="bc_ps")
            nc.tensor.matmul(nb_ps[:], ones_row[:], ngmax[:],
                             start=True, stop=True)
            nbias = small.tile([P, 1], fp32, tag="nbias")
            nc.scalar.copy(nbias[:], nb_ps[:])

            p_sb = epers.tile([P, NCOL], fp32, tag=f"p{bi}")
            ssum = small.tile([P, 1], fp32, tag="ssum")
            nc.scalar.activation(p_sb[:], e_sb[:], Act.Exp,
                                 bias=nbias[:], scale=1.0, accum_out=ssum[:])

            tot_ps = psum.tile([1, 1], fp32, tag="red_ps")
            nc.tensor.matmul(tot_ps[:], ones_col[:], ssum[:],
                             start=True, stop=True)
            rinv = small.tile([1, 1], fp32, tag="rinv")
            nc.vector.reciprocal(rinv[:], tot_ps[:])
            r_ps = psum.tile([P, 1], fp32, tag="bc_ps")
            nc.tensor.matmul(r_ps[:], ones_row[:], rinv[:],
                             start=True, stop=True)
            rbc = small.tile([P, 1], fp32, tag="rbc")
            nc.scalar.copy(rbc[:], r_ps[:])

            o_sb = epers.tile([P, NCOL], fp32, tag=f"o{bi}")
            nc.scalar.mul(o_sb[:], p_sb[:], rbc[:])

            # o_sb[p, (c,k)] holds l = c*512 + p*4 + k: store directly as
            # [128, 8, 4] -> per-partition 8 runs of 16 contiguous bytes.
            nc.scalar.dma_start(
                probs[bi].rearrange("(c p k) -> p c k", p=P, k=JCH),
                o_sb[:].rearrange("p (c k) -> p c k", k=JCH),
            )

    nc.compile()
    return nc


def _get_program():
    global _PROGRAM
    if _PROGRAM is None:
        _PROGRAM = _build_program()
    return _PROGRAM


def kernel(hidden, encoder_outputs, W, b):
    """Full-input entry point: shards across 8 NeuronCores, returns [B,1,L]."""
    from concourse.bass_utils import run_bass_kernel_spmd

    hidden = np.asarray(hidden, dtype=np.float32)
    enc = np.asarray(encoder_outputs, dtype=np.float32)
    W = np.asarray(W, dtype=np.float32)

    h_last = hidden[0, 1]  # == hidden[0].transpose(1,0,2)[:, -1, :], [B, H]

    nc = _get_program()
    in_maps = []
    for core in range(N_CORES):
        b0 = core * B_LOC
        in_maps.append({
            "enc": np.ascontiguousarray(enc[b0:b0 + B_LOC]),
            "hT": np.ascontiguousarray(h_last[b0:b0 + B_LOC].T),
            "W": np.ascontiguousarray(W),
        })

    res = run_bass_kernel_spmd(nc, in_maps, list(range(N_CORES)))
    out = np.concatenate([res.results[i]["probs"] for i in range(N_CORES)], axis=0)
    return out[:, None, :].astype(np.float32)


# revision 13
# speedup vs baseline: 1.0545x; 1.0448x over previous
"""Trainium2 Bass kernel for nn_Attention_43181601194684.

Reference computation:
    h_last  = hidden[0, 1]                          # [B, H]
    proj    = einsum('blh,oh->blo', enc, W) + b     # [B, L, H]
    energies= einsum('bh,blh->bl', h_last, proj)    # [B, L]
    out     = softmax(energies, axis=1)[:, None, :] # [B, 1, L]

Algebraic simplification used here:
    energies[b, l] = (h_last[b] @ W) . enc[b, l] + (h_last[b] . bias)
The per-batch constant cancels inside the softmax, so the device kernel
computes   e[b, l] = v[b] . enc[b, l]   with v = h_last @ W, followed by a
numerically-stable softmax over l.

Sharding: data-parallel over batch. 32 batches / 8 cores = 4 batches per
core; W is replicated; the [2,2,32,512] hidden tensor is sliced to the
4 x 512 h_last rows each core needs (passed pre-transposed as [512, 4]).
"""

import numpy as np

B, L, H = 32, 4096, 512
N_CORES = 8
B_LOC = B // N_CORES  # 4
P = 128               # SBUF partitions
JCH = 4               # 128-row l-blocks per DMA chunk (1 MiB per DMA)
NCH = L // (P * JCH)  # 8 chunks per batch
NCOL = L // P         # 32 energy columns per batch

_PROGRAM = None


def _build_program():
    """Build + compile the single-core Bass/Tile program (SPMD across 8 cores)."""
    from contextlib import ExitStack

    import concourse.bacc as bacc
    import concourse.mybir as mybir
    import concourse.tile as tile
    from concourse.masks import make_identity

    fp32 = mybir.dt.float32
    Alu = mybir.AluOpType
    Act = mybir.ActivationFunctionType

    nc = bacc.Bacc("TRN2", target_bir_lowering=False, debug=False,
                   num_devices=N_CORES)

    enc = nc.dram_tensor("enc", [B_LOC, L, H], fp32, kind="ExternalInput")
    h4 = nc.dram_tensor("h4", [B_LOC, H], fp32, kind="ExternalInput")
    Wd = nc.dram_tensor("W", [H, H], fp32, kind="ExternalInput")
    probs = nc.dram_tensor("probs", [B_LOC, L], fp32, kind="ExternalOutput")

    with tile.TileContext(nc) as tc, ExitStack() as ctx:
        consts = ctx.enter_context(tc.tile_pool(name="consts", bufs=1))
        wpool = ctx.enter_context(tc.tile_pool(name="wpool", bufs=1))
        epool = ctx.enter_context(tc.tile_pool(name="epool", bufs=4))
        scratch = ctx.enter_context(tc.tile_pool(name="scratch", bufs=2))
        epers = ctx.enter_context(tc.tile_pool(name="epers", bufs=1))
        small = ctx.enter_context(tc.tile_pool(name="small", bufs=2))
        psum = ctx.enter_context(tc.tile_pool(name="psum", bufs=2, space="PSUM"))

        identity = consts.tile([P, P], fp32, tag="identity")
        make_identity(nc, identity)
        ones_row = consts.tile([1, P], fp32, tag="ones_row")  # bcast lhsT
        nc.vector.memset(ones_row[:], 1.0)
        ones_col = consts.tile([P, 1], fp32, tag="ones_col")  # partition-sum lhsT
        nc.vector.memset(ones_col[:], 1.0)

        # ---- v = h_last @ W  (PE, contraction over o in 4 chunks of 128) ----
        # W/h4 ride the scalar HWDGE ring; h4 loads naturally ([4, 512]
        # contiguous) and is transposed on the PE into [128, 4] lhsT chunks.
        w_r = Wd.rearrange("(c p) h -> c p h", p=P)
        w_tiles = []
        for c in range(4):
            wt = wpool.tile([P, H], fp32, tag=f"w{c}")
            nc.scalar.dma_start(wt[:], w_r[c])
            w_tiles.append(wt)
        h4_sb = wpool.tile([B_LOC, H], fp32, tag="h4")
        nc.scalar.dma_start(h4_sb[:], h4[:])
        h_tiles = []
        for c in range(4):
            htr_ps = psum.tile([P, B_LOC], fp32, tag="bc_ps")
            nc.tensor.transpose(htr_ps[:], h4_sb[:, c * P:(c + 1) * P],
                                identity[:B_LOC, :B_LOC])
            ht = wpool.tile([P, B_LOC], fp32, tag=f"h{c}")
            nc.scalar.copy(ht[:], htr_ps[:])
            h_tiles.append(ht)

        v_ps = psum.tile([B_LOC, H], fp32, tag="mm_ps")
        for c in range(4):
            nc.tensor.matmul(v_ps[:], h_tiles[c][:], w_tiles[c][:],
                             start=(c == 0), stop=(c == 3))
        v_sb = wpool.tile([B_LOC, H], fp32, tag="v_sb")
        nc.scalar.copy(v_sb[:], v_ps[:])

        # ---- broadcast v[b] to all 128 partitions via one-hot PE matmul ----
        # lhsT[k, m] = (k == bi)  =>  out[m, :] = v_sb[bi, :] for every m
        v_bc = []
        for bi in range(B_LOC):
            sel = consts.tile([B_LOC, P], fp32, tag=f"sel{bi}")
            nc.gpsimd.memset(sel[:], 0.0)
            # iota = 1*partition + 0*free - bi; != 0 keeps 0.0, == 0 fills 1.0
            nc.gpsimd.affine_select(
                out=sel[:], in_=sel[:], compare_op=Alu.not_equal, fill=1.0,
                base=-bi, pattern=[[0, P]], channel_multiplier=1,
            )
            vb_ps = psum.tile([P, H], fp32, tag="mm_ps")
            nc.tensor.matmul(vb_ps[:], sel[:], v_sb[:],
                             start=True, stop=True)
            vb = wpool.tile([P, H], fp32, tag=f"vbc{bi}")
            nc.scalar.copy(vb[:], vb_ps[:])
            v_bc.append(vb)

        # ---- main stream: energies via fused multiply+row-reduce on DVE ----
        # l = c*512 + p*4 + k: partition p owns 4 consecutive rows = one
        # contiguous 8 KiB DRAM run per partition -> long DMA descriptors.
        enc_r = enc.rearrange("b (c p k) h -> b c p k h", p=P, k=JCH)

        for bi in range(B_LOC):
            e_sb = epers.tile([P, NCOL], fp32, tag=f"e{bi}")
            for c in range(NCH):
                et = epool.tile([P, JCH, H], fp32, tag="et")
                # alternate the two HWDGE rings so SDMA engines always have
                # a packet queued on a second ring while one completes
                dma_eng = nc.sync if (c % 2 == 0) else nc.scalar
                dma_eng.dma_start(et[:], enc_r[bi, c])
                for k in range(JCH):
                    m = c * JCH + k
                    sc = scratch.tile([P, H], fp32, tag="ttr")
                    # fused (enc * v) + row-sum in one native DVE op:
                    # out = (in0 * 1.0) * in1 ; accum_out = row_sum(out)
                    nc.vector.scalar_tensor_tensor(
                        out=sc[:], in0=et[:, k, :], scalar=1.0,
                        in1=v_bc[bi][:],
                        op0=Alu.mult, op1=Alu.mult,
                        accum_out=e_sb[:, m:m + 1],
                    )

            # ---- softmax over the 4096 energies of batch bi ----
            mx = small.tile([P, 1], fp32, tag="mx")
            nc.vector.tensor_reduce(mx[:], e_sb[:], axis=mybir.AxisListType.X,
                                    op=Alu.max)
            mxT_ps = psum.tile([1, P], fp32, tag="red_ps")
            nc.tensor.transpose(mxT_ps[:], mx[:], identity[:])
            ngmax = small.tile([1, 1], fp32, tag="ngmax")
            nc.vector.tensor_reduce(ngmax[:], mxT_ps[:],
                                    axis=mybir.AxisListType.X, op=Alu.max,
                                    negate=True)
            nb_ps = psum.tile([P, 1], fp32, tag="bc_ps")
            nc.tensor.matmul(nb_ps[:], ones_row[:], ngmax[:],
                             start=True, stop=True)
            nbias = small.tile([P, 1], fp32, tag="nbias")
            nc.scalar.copy(nbias[:], nb_ps[:])

            p_sb = epers.tile([P, NCOL], fp32, tag=f"p{bi}")
            ssum = small.tile([P, 1], fp32, tag="ssum")
            nc.scalar.activation(p_sb[:], e_sb[:], Act.Exp,
                                 bias=nbias[:], scale=1.0, accum_out=ssum[:])

            tot_ps = psum.tile([1, 1], fp32, tag="red_ps")
            nc.tensor.matmul(tot_ps[:], ones_col[:], ssum[:],
                             start=True, stop=True)
            rinv = small.tile([1, 1], fp32, tag="rinv")
            nc.vector.reciprocal(rinv[:], tot_ps[:])
            r_ps = psum.tile([P, 1], fp32, tag="bc_ps")
            nc.tensor.matmul(r_ps[:], ones_row[:], rinv[:],
                             start=True, stop=True)
            rbc = small.tile([P, 1], fp32, tag="rbc")
            nc.scalar.copy(rbc[:], r_ps[:])

            o_sb = epers.tile([P, NCOL], fp32, tag=f"o{bi}")
            nc.scalar.mul(o_sb[:], p_sb[:], rbc[:])

            # o_sb[p, (c,k)] holds l = c*512 + p*4 + k: store directly as
            # [128, 8, 4] -> per-partition 8 runs of 16 contiguous bytes.
            nc.scalar.dma_start(
                probs[bi].rearrange("(c p k) -> p c k", p=P, k=JCH),
                o_sb[:].rearrange("p (c k) -> p c k", k=JCH),
            )

    nc.compile()
    return nc


def _get_program():
    global _PROGRAM
    if _PROGRAM is None:
        _PROGRAM = _build_program()
    return _PROGRAM


def kernel(hidden, encoder_outputs, W, b):
    """Full-input entry point: shards across 8 NeuronCores, returns [B,1,L]."""
    from concourse.bass_utils import run_bass_kernel_spmd

    hidden = np.asarray(hidden, dtype=np.float32)
    enc = np.asarray(encoder_outputs, dtype=np.float32)
    W = np.asarray(W, dtype=np.float32)

    h_last = hidden[0, 1]  # == hidden[0].transpose(1,0,2)[:, -1, :], [B, H]

    nc = _get_program()
    in_maps = []
    for core in range(N_CORES):
        b0 = core * B_LOC
        in_maps.append({
            "enc": np.ascontiguousarray(enc[b0:b0 + B_LOC]),
            "h4": np.ascontiguousarray(h_last[b0:b0 + B_LOC]),
            "W": np.ascontiguousarray(W),
        })

    res = run_bass_kernel_spmd(nc, in_maps, list(range(N_CORES)))
    out = np.concatenate([res.results[i]["probs"] for i in range(N_CORES)], axis=0)
    return out[:, None, :].astype(np.float32)
